# revision 9
# baseline (speedup 1.0000x reference)
"""Self-attention kernel for Trainium2, SPMD across 8 NeuronCores.

Reference computation (fp32):
    q = x @ Wq + bq; k = x @ Wk + bk; v = x @ Wv + bv
    out = softmax((q @ k.T) / sqrt(d_q), axis=1) @ v

Sharding: rows of Q (sequence dim N=8192) are sharded across the 8
cores (1024 rows each).  K/V for the full sequence come from a hybrid
of redundant local projection and one ncfw AllGather:

  - every core projects its OWN 1024 tokens' K/V (the "shard") and
    bounces it to DRAM, then fires an AllGather ([2,128,1024] ->
    [16,128,1024] bf16, rank-major);
  - every core also redundantly projects ABSOLUTE token blocks 0..7
    (the same set on every core -- the SPMD program must be identical
    across cores, and softmax key order is permutation-invariant, so
    SBUF key-slot assignment need not follow the global order);
  - absolute blocks 8..15 (= AG ranks 4..7) are NOT projected locally;
    their K^T/V arrive via the AllGather and are DMA'd straight into
    the same SBUF tensors (2 DMAs per rank, 2KB contiguous lines).

The AG in this environment costs ~25us control floor + ~35us data, so
it is fired at ~20us (as soon as the local shard exists) and lands at
~80us -- hidden behind the ~60us of redundant projection + attention
on blocks 0..7.  Only the last 8 blocks' attention waits on it.
Against the all-redundant baseline this removes 6 of 16 block
projections (~27us of PE) and ~4.5MB of x DMA traffic.

All matmul operands are bfloat16 (1 cyc/row on the PE, FWL fast
weight loads) with fp32 PSUM accumulation; PSUM matmuls are 512 wide
(bank-boundary limit).  The softmax denominator accumulates on DVE in
bf16 (2x the fp32 DVE rate; the per-element rounding error averages
out over the 128-partition epilogue sum -- ~0.1% on the denominator).

Per-core dataflow:
  - ~4us of dummy matmuls during the initial DMA wait pre-warm the PE
    HAM clock gate to 2.4GHz
  - own blocks: K^T[dk,tok], V^T -> V natural (PE transpose), Q^T;
    biases added during the DVE PSUM->SBUF eviction so the gathered
    shards arrive fully biased; bounce + AllGather on the sync queue
    (idle mid-kernel) so no compute engine stalls on collective waits
  - redundant blocks -> kT_sb/v_sb slots 0..7, as the baseline
  - per j-tile (128 keys): S^T[kj,qi] = K_tile^T.T @ Q^T; one
    [128,1024] exp on ACT (scale=1/sqrt(128), no max subtraction --
    |scores| < ~3); bf16 denominator accumulation on DVE;
    O^T[dv,qi] += V_tile.T @ E accumulated in PSUM across all 64
    j-tiles, V-matmuls one j-tile behind the S-matmuls so the
    in-order PE never stalls on exp
  - epilogue: denominator partition-sum via bf16 ones-matmuls, DVE
    reciprocal, O^T transposed back 128 rows at a time with the 1/den
    scale fused into the DVE eviction.
"""

import numpy as np

import concourse.bacc as bacc
import concourse.mybir as mybir
import concourse.tile as tile
from concourse.bass_utils import run_bass_kernel_spmd
from concourse.masks import make_identity

N_CORES = 8
N = 8192          # sequence length
D = 1024          # d_model
DH = 128          # d_q == d_k == d_v
NB = N // N_CORES # tokens per core (1024)
KT = D // 128     # k-tiles in the contraction over d_model (8)
JBLK = 512        # token block for the K/V projection stream
NJB = N // JBLK   # 16 key blocks total
NRED = 8          # absolute blocks 0..7 projected redundantly on every core
NAGR = (NJB - NRED) // 2  # ranks delivered by the AllGather (4: ranks 4..7)
NJT = N // 128    # 64 j-tiles in the attention loop
QBLK = 512        # query block (PSUM bank limit)
NQB = NB // QBLK  # 2
FB = KT * JBLK    # 4096 floats per partition per stream block

F32 = mybir.dt.float32
BF16 = mybir.dt.bfloat16
SCALE = 1.0 / float(np.sqrt(DH))

_CACHE = {}

# Results of the last run_bass_kernel_spmd call (for the test harness to
# read exec_time_ns etc. when tracing is enabled via BASS_TRACE).
LAST_RESULTS = None


def _emit(ctx, tc, nc, xT_own, xT_red, w_all, b_all, out):
    singles = ctx.enter_context(tc.tile_pool(name="singles", bufs=1))
    dram = ctx.enter_context(tc.tile_pool(name="dram", bufs=1, space="DRAM"))
    xt_pool = ctx.enter_context(tc.tile_pool(name="xt", bufs=6))
    vt_pool = ctx.enter_context(tc.tile_pool(name="vt", bufs=3))
    exp_pool = ctx.enter_context(tc.tile_pool(name="exp", bufs=6))
    oT_pool = ctx.enter_context(tc.tile_pool(name="oT", bufs=3))
    o_pool = ctx.enter_context(tc.tile_pool(name="o", bufs=3))
    ps_pool = ctx.enter_context(tc.tile_pool(name="ps", bufs=2, space="PSUM"))
    pp_pool = ctx.enter_context(tc.tile_pool(name="pp", bufs=2, space="PSUM"))
    po_pool = ctx.enter_context(tc.tile_pool(name="po", bufs=1, space="PSUM"))

    # --- constants / weights ---------------------------------------------
    # w_all layout is (Wk | Wv | Wq); K+V land first in a 0.5MB DMA so the
    # first stream block's projections start as early as possible.
    w_sb = singles.tile([128, 3 * D], BF16, tag="w_sb")
    nc.sync.dma_start(out=w_sb[:, 0:2 * D], in_=w_all[:, 0:2 * D])
    nc.sync.dma_start(out=w_sb[:, 2 * D:3 * D], in_=w_all[:, 2 * D:3 * D])
    b_sb = singles.tile([128, 3], F32, tag="b_sb")
    nc.sync.dma_start(out=b_sb, in_=b_all)
    ident_bf = singles.tile([128, 128], BF16, tag="ident_bf")
    ones128 = singles.tile([128, 1], BF16, tag="ones128")
    nc.vector.memset(ones128, 1.0)

    W_BASE = {1: 0, 2: D, 0: 2 * D}  # k, v, q order in w_all

    def w_ap(proj, kt):  # lhsT [128, 128] for projection matmuls
        base = W_BASE[proj] + kt * 128
        return w_sb[:, base:base + 128]

    # --- persistent SBUF tensors -----------------------------------------
    kT_sb = singles.tile([128, N], BF16, tag="kT")    # K^T, all key slots
    v_sb = singles.tile([128, N], BF16, tag="v")      # V natural, 64 j-tiles
    own_kT = singles.tile([128, NB], BF16, tag="own_kT")  # own shard K^T
    own_v = singles.tile([128, NB], BF16, tag="own_v")    # own shard V
    qT_sb = singles.tile([128, NB], BF16, tag="qT")   # Q^T, local tokens
    rden_sb = singles.tile([128, NB // 128], F32, tag="rden")
    acc_all = singles.tile([128, NB], BF16, tag="acc_all", name="acc_all")
    po_t = po_pool.tile([128, NB], F32, tag="po", name="po_t")

    bounce_in = dram.tile([2, 128, NB], BF16, tag="bounce_in")
    bounce_out = dram.tile([2 * N_CORES, 128, NB], BF16, tag="bounce_out",
                           addr_space="Shared")

    def project(parts, kT_dst, v_dst, q_dst):
        """Project one 512-token block: K^T/V(+bias), optionally Q^T."""
        def xsl(kt):
            for t, base in reversed(parts):
                if kt >= base:
                    return t[:, (kt - base) * JBLK:(kt - base + 1) * JBLK]

        ps_k = pp_pool.tile([128, JBLK], F32, tag="pp")
        for kt in range(KT):
            nc.tensor.matmul(ps_k, w_ap(1, kt), xsl(kt),
                             start=(kt == 0), stop=(kt == KT - 1))
        nc.vector.tensor_scalar_add(kT_dst, ps_k, b_sb[:, 1:2])

        ps_v = pp_pool.tile([128, JBLK], F32, tag="pp")
        for kt in range(KT):
            nc.tensor.matmul(ps_v, w_ap(2, kt), xsl(kt),
                             start=(kt == 0), stop=(kt == KT - 1))
        vT_t = vt_pool.tile([128, JBLK], BF16, tag="vt")
        nc.vector.tensor_scalar_add(vT_t, ps_v, b_sb[:, 2:3])
        for c in range(4):
            ps_tp = pp_pool.tile([128, 512], BF16, tag="pp")
            dst = ps_tp[:, 0:128]
            nc.tensor.transpose(dst, vT_t[:, c * 128:(c + 1) * 128], ident_bf)
            nc.vector.tensor_copy(v_dst[:, c * 128:(c + 1) * 128], dst)

        if q_dst is not None:
            ps_q = pp_pool.tile([128, JBLK], F32, tag="pp")
            for kt in range(KT):
                nc.tensor.matmul(ps_q, w_ap(0, kt), xsl(kt),
                                 start=(kt == 0), stop=(kt == KT - 1))
            nc.scalar.activation(out=q_dst, in_=ps_q,
                                 func=mybir.ActivationFunctionType.Identity,
                                 bias=b_sb[:, 0:1], scale=1.0)

    def stream_own(h):
        """Own half-shard h: K^T/V -> own_kT/own_v, Q^T -> qT_sb."""
        if h == 0:
            ha = xt_pool.tile([128, FB // 2], BF16, tag="xt", name="xo0a")
            nc.gpsimd.dma_start(out=ha, in_=xT_own[0, :, 0:FB // 2])
            hb = xt_pool.tile([128, FB // 2], BF16, tag="xt", name="xo0b")
            nc.gpsimd.dma_start(out=hb, in_=xT_own[0, :, FB // 2:FB])
            # identities built here: after block 0's DMA issues (so they
            # don't delay them on gpsimd) but before any transpose reads
            make_identity(nc, ident_bf)
            parts = ((ha, 0), (hb, KT // 2))
        else:
            xt_t = xt_pool.tile([128, FB], BF16, tag="xt", name="xo1")
            nc.gpsimd.dma_start(out=xt_t, in_=xT_own[1])
            parts = ((xt_t, 0),)
        tok = slice(h * JBLK, (h + 1) * JBLK)
        project(parts, own_kT[:, tok], own_v[:, tok],
                qT_sb[:, h * JBLK:(h + 1) * JBLK])

    def stream_red(i):
        """Redundant absolute block i -> kT_sb/v_sb slot i."""
        xt_t = xt_pool.tile([128, FB], BF16, tag="xt", name=f"xr{i}")
        nc.gpsimd.dma_start(out=xt_t, in_=xT_red[i])
        parts = ((xt_t, 0),)
        tok = slice(i * JBLK, (i + 1) * JBLK)
        project(parts, kT_sb[:, tok], v_sb[:, tok], None)

    def exchange_local():
        """Bounce the own K^T/V shard to DRAM and fire the AllGather.

        On the sync queue: it is idle mid-kernel, so the collective's
        input-ready wait stalls nothing else."""
        nc.sync.dma_start(out=bounce_in[0], in_=own_kT)
        nc.sync.dma_start(out=bounce_in[1], in_=own_v)
        nc.gpsimd.collective_compute(
            "AllGather",
            mybir.AluOpType.bypass,
            replica_groups=[list(range(N_CORES))],
            ins=[bounce_in.opt()],
            outs=[bounce_out.opt()],
        )

    def read_remote():
        """AG ranks 4..7 -> key slots 8..15 (uniform across cores)."""
        for r in range(N_CORES - NAGR, N_CORES):
            sl = slice(2 * r * JBLK, 2 * r * JBLK + NB)
            nc.sync.dma_start(out=kT_sb[:, sl], in_=bounce_out[2 * r])
            nc.sync.dma_start(out=v_sb[:, sl], in_=bounce_out[2 * r + 1])

    # The V-matmuls run one j-tile behind the S-matmuls (software
    # pipeline): the in-order PE then never stalls on exp(jt) -- V(jt-1)
    # executes while ACT computes exp(jt).
    pend = []

    def emit_v(jt, e):
        kj = slice(jt * 128, (jt + 1) * 128)
        for qb in range(NQB):
            qs = slice(qb * QBLK, (qb + 1) * QBLK)
            nc.tensor.matmul(po_t[:, qs], v_sb[:, kj], e[:, qs],
                             start=(jt == 0), stop=(jt == NJT - 1))

    def attention_block(jb):
        for c in range(4):
            jt = jb * 4 + c
            kj = slice(jt * 128, (jt + 1) * 128)
            ps_s = ps_pool.tile([128, NB], F32, tag="ps")
            for qb in range(NQB):
                qs = slice(qb * QBLK, (qb + 1) * QBLK)
                nc.tensor.matmul(ps_s[:, qs], kT_sb[:, kj], qT_sb[:, qs],
                                 start=True, stop=True)
            e = exp_pool.tile([128, NB], BF16, tag="exp")
            nc.scalar.activation(out=e, in_=ps_s,
                                 func=mybir.ActivationFunctionType.Exp,
                                 scale=SCALE)
            if jt == 0:
                nc.vector.tensor_copy(acc_all, e)
            else:
                nc.vector.tensor_add(acc_all, acc_all, e)
            if pend:
                emit_v(*pend.pop())
            pend.append((jt, e))

    # --- PE warm-up -------------------------------------------------------
    # ~4us of dummy matmuls during the initial DMA wait flips the PE HAM
    # clock gate to 8/8 before the real work arrives (PE is idle anyway).
    warm = singles.tile([128, 512], BF16, tag="warm")
    nc.vector.memset(warm, 0.0)
    ps_w = ps_pool.tile([128, NB], F32, tag="ps")
    for _ in range(20):
        nc.tensor.matmul(ps_w[:, 0:512], warm[:, 0:128], warm,
                         start=True, stop=True)

    # --- main stream ------------------------------------------------------
    stream_own(0)
    stream_own(1)
    stream_red(0)
    stream_red(1)
    attention_block(0)
    for i in range(2, NRED):
        stream_red(i)
        attention_block(i - 1)
    # The collective trigger lives on the gpsimd queue and waits for the
    # bounce-in DMAs; issued here (after every xT stream DMA is already
    # in the queue) so the wait cannot stall the x stream.
    exchange_local()
    read_remote()
    attention_block(NRED - 1)
    for jb in range(NRED, NJB):
        attention_block(jb)
    emit_v(*pend.pop())  # flush the pipelined last V-matmul

    # --- epilogue ---------------------------------------------------------
    # denominator: sum acc over its 128 partitions via ones-matmuls, one
    # [128,1] chunk per 128 queries (lands per-partition).  acc is bf16
    # throughout, so the matmuls run single-pass.
    NG = NB // 128
    ps_d = ps_pool.tile([128, NB], F32, tag="ps")
    for g in range(NG):
        nc.tensor.matmul(ps_d[:, g:g + 1],
                         acc_all[:, g * 128:(g + 1) * 128], ones128,
                         start=True, stop=True)
    nc.vector.reciprocal(rden_sb, ps_d[:, 0:NG])
    for g in range(NB // 128):
        # O^T -> SBUF (bf16, per 128-query chunk), transpose (1 cyc/row),
        # scale by 1/den on DVE, store.  Chunked copies + per-chunk PSUM
        # tiles let the transpose->scale->store chains pipeline.
        oT_t = oT_pool.tile([128, 128], BF16, tag="oT")
        nc.vector.tensor_copy(oT_t, po_t[:, g * 128:(g + 1) * 128])
        ps_to = pp_pool.tile([128, 512], BF16, tag="pp")
        dst = ps_to[:, 0:128]
        nc.tensor.transpose(dst, oT_t, ident_bf)
        ob = o_pool.tile([128, DH], F32, tag="o")
        nc.vector.tensor_scalar_mul(ob, dst, rden_sb[:, g:g + 1])
        nc.sync.dma_start(out=out[g * 128:(g + 1) * 128, :], in_=ob)


def build_nc():
    if "nc" in _CACHE:
        return _CACHE["nc"]
    from contextlib import ExitStack

    nc = bacc.Bacc("TRN2", target_bir_lowering=False, debug=False,
                   num_devices=N_CORES)
    xT_own = nc.dram_tensor("xT_own", [2, 128, FB], BF16,
                            kind="ExternalInput").ap()
    xT_red = nc.dram_tensor("xT_red", [NRED, 128, FB], BF16,
                            kind="ExternalInput").ap()
    w_all = nc.dram_tensor("w_all", [128, 3 * D], BF16, kind="ExternalInput").ap()
    b_all = nc.dram_tensor("b_all", [128, 3], F32, kind="ExternalInput").ap()
    out = nc.dram_tensor("out", [NB, DH], F32, kind="ExternalOutput").ap()

    with tile.TileContext(nc) as tc:
        with ExitStack() as ctx:
            _emit(ctx, tc, nc, xT_own, xT_red, w_all, b_all, out)
    nc.compile()
    _CACHE["nc"] = nc
    return nc


def make_in_maps(inputs):
    x = np.asarray(inputs["x"], dtype=np.float32)
    # blocked x.T: blk[jb, p, kt*JBLK + n] = x.T[kt*128 + p, jb*JBLK + n]
    #            = x[jb*JBLK + n, kt*128 + p]
    import ml_dtypes
    xb = x.reshape(NJB, JBLK, KT, 128)                    # [jb, n, kt, p]
    blk = np.ascontiguousarray(
        xb.transpose(0, 3, 2, 1)).reshape(NJB, 128, FB).astype(
        ml_dtypes.bfloat16)                               # [jb, p, kt*n]
    xT_red = np.ascontiguousarray(blk[:NRED])             # abs blocks 0..7

    w_cols = []
    for wn in ("Wk", "Wv", "Wq"):
        w = np.asarray(inputs[wn], np.float32)            # [D, DH]
        wr = w.reshape(KT, 128, DH).transpose(1, 0, 2).reshape(128, D)
        w_cols.append(wr)
    w_all = np.concatenate(w_cols, axis=1).astype(ml_dtypes.bfloat16)
    b_all = np.ascontiguousarray(np.stack(
        [np.asarray(inputs[bn], np.float32) for bn in ("bq", "bk", "bv")],
        axis=1))                                          # [128, 3]

    in_maps = []
    for c in range(N_CORES):
        m = {
            "xT_own": np.ascontiguousarray(blk[2 * c:2 * c + 2]),
            "xT_red": xT_red,
            "w_all": w_all,
            "b_all": b_all,
        }
        in_maps.append(m)
    return in_maps


def kernel(**inputs) -> np.ndarray:
    global LAST_RESULTS
    nc = build_nc()
    in_maps = make_in_maps(inputs)
    res = run_bass_kernel_spmd(nc, in_maps, core_ids=list(range(N_CORES)))
    LAST_RESULTS = res
    return np.concatenate([res.results[c]["out"] for c in range(N_CORES)],
                          axis=0)
